# revision 5
# baseline (speedup 1.0000x reference)
"""Trainium2 Bass kernel for nn_Loss_Labels_19825569038545.

Computes -mean(log_softmax(concat([syn, ant], axis=1), axis=1)) over
B=2^24 rows.

Math: per row with s=syn, a=ant, d=s-a:
    -(lsm_0 + lsm_1) = softplus(-d) + softplus(d) = 2*ln(1+e^{-d}) + d
so   loss = (2*sum_i ln(1+u_i) + sum_i d_i) / (2B),   u = e^{-d}.

The ln work is cut 4x by pairing: ln(1+u1)+ln(1+u2)+ln(1+u3)+ln(1+u4)
= ln((1+u1)(1+u2)(1+u3)(1+u4)); the products are cheap DVE multiplies,
so the ACT engine runs one full-rate Exp pass and a quarter-rate Ln
pass instead of two full-rate passes.

Inputs are staged to the device as bfloat16 (the 2e-2 correctness gate
leaves ~3 decimal digits of headroom; measured end-to-end rel-err of
the bf16 pipeline is ~2e-5), which halves HBM traffic: each of 8 cores
streams an 8 MiB shard. Per tile [128, 2w]:

    DVE  tensor_tensor_reduce: d = s - a   (bf16, fused accum -> Σd)
    ACT  Exp(scale=-1):        u = e^{-d}  (in place, bf16)
    DVE  tensor_scalar_add:    v = 1 + u   (in place)
    DVE  mult: p = v_lo * v_hi;  q = p_lo * p_hi   ([128, w/4] products)
    ACT  Ln(q) with fused accum -> Σln

Each core writes [128, 2*nt] fp32 per-partition partials; the host
combines them in float64. S1_out only provides B and is never
transferred. Raw Bass (no TileContext); explicit semaphores.
"""

import sys
from contextlib import ExitStack

import numpy as np

try:
    import concourse.bass  # noqa: F401
except ImportError:
    sys.path.insert(0, "/opt/trn_rl_repo")

import ml_dtypes
import concourse.bass as bass
import concourse.mybir as mybir
from concourse.bass_utils import run_bass_kernel_spmd

B = 16777216
N_CORES = 8
N = B // N_CORES          # 2,097,152 elements per core
P = 128                   # SBUF partitions
WTOT = N // P             # 16384 columns per partition per core

# Tile widths (columns): sum must be WTOT, each divisible by 8 (the
# pairing splits a tile into quarters that must stay 4B-aligned for the
# DVE's packed 16-bit mode). Narrow head for fast pipeline fill, narrow
# tail for a short drain chain.
WIDTHS = (512, 1024, 2048, 4096, 4096, 2048, 1024, 1024, 512)

FP32 = mybir.dt.float32
BF16 = mybir.dt.bfloat16
ALU = mybir.AluOpType

_nc_cache = {}


def _build_nc(widths=WIDTHS, repeat=1, distinct=1):
    """Build the per-core program. repeat/distinct are benchmarking knobs
    (replay the streaming pass over `distinct` separate input regions);
    repeat=1, distinct=1 is the graded kernel."""
    widths = tuple(widths)
    assert sum(widths) == WTOT
    assert all(w % 8 == 0 for w in widths)
    nt = len(widths)
    offs = np.concatenate([[0], np.cumsum(widths)]).tolist()
    key = (widths, repeat, distinct)
    if key in _nc_cache:
        return _nc_cache[key]
    nc = bass.Bass()
    # Flat DRAM layout: tile i is a contiguous [P, 2*w] bf16 slab at
    # element offset P*2*offs[i]; within it each partition row holds the
    # syn chunk then the ant chunk.
    sa = nc.dram_tensor("sa", [distinct, P * 2 * WTOT], BF16, kind="ExternalInput")
    # col i = Σd of tile i, col nt+i = Σln of tile i (host sums).
    out = nc.dram_tensor("out", [P, 2 * nt], FP32, kind="ExternalOutput")

    with ExitStack() as ctx:
        sa_tiles = [
            ctx.enter_context(nc.sbuf_tensor(f"sa_t{i}", [P, 2 * w], BF16))
            for i, w in enumerate(widths)
        ]
        w_tiles = [
            ctx.enter_context(nc.sbuf_tensor(f"w_t{i}", [P, w], BF16))
            for i, w in enumerate(widths)
        ]
        acc = ctx.enter_context(nc.sbuf_tensor("acc", [P, 2 * nt], FP32))
        load_sems = [
            ctx.enter_context(nc.semaphore(f"load{i}")) for i in range(nt)
        ]
        dve_pipe = ctx.enter_context(nc.semaphore("dve_pipe"))
        act_pipe = ctx.enter_context(nc.semaphore("act_pipe"))
        st_sem = ctx.enter_context(nc.semaphore("store_done"))
        block = ctx.enter_context(nc.Block())

        # Software-pipelined emission: DVE issues ttr(i+1) before the
        # post-exp ops of tile i, and ACT issues exp(i+1) before ln(i),
        # so neither engine idles inside the per-tile DVE->ACT->DVE chain.
        # Sem values are assigned in emission order; these tables hold the
        # post-inc value of each logical op for cross-engine waits.
        ttr_seq = np.zeros((repeat, nt), dtype=int)
        ts_seq = np.zeros((repeat, nt), dtype=int)
        p_seq = np.zeros((repeat, nt), dtype=int)
        q_seq = np.zeros((repeat, nt), dtype=int)
        exp_seq = np.zeros((repeat, nt), dtype=int)
        ln_seq = np.zeros((repeat, nt), dtype=int)

        def dve_order():
            """(kind, r, i) in DVE emission order: ttr runs 1 tile ahead."""
            k = 0  # global tile counter across passes
            flat = [(r, i) for r in range(repeat) for i in range(nt)]
            yield ("ttr", *flat[0])
            for k, (r, i) in enumerate(flat):
                if k + 1 < len(flat):
                    yield ("ttr", *flat[k + 1])
                yield ("ts", r, i)
                yield ("p", r, i)
                yield ("q", r, i)

        def act_order():
            flat = [(r, i) for r in range(repeat) for i in range(nt)]
            yield ("exp", *flat[0])
            for k, (r, i) in enumerate(flat):
                if k + 1 < len(flat):
                    yield ("exp", *flat[k + 1])
                yield ("ln", r, i)

        seqs = {"ttr": ttr_seq, "ts": ts_seq, "p": p_seq, "q": q_seq}
        n = 0
        for kind, r, i in dve_order():
            n += 1
            seqs[kind][r, i] = n
        n = 0
        for kind, r, i in act_order():
            n += 1
            (exp_seq if kind == "exp" else ln_seq)[r, i] = n

        @block.sync
        def _(sync):
            for r in range(repeat):
                d_idx = r % distinct
                for i, w in enumerate(widths):
                    if r > 0:
                        # sa_tiles[i] consumed once ttr(r-1, i) is done
                        sync.wait_ge(dve_pipe, ttr_seq[r - 1, i])
                    base = P * 2 * offs[i]
                    sync.dma_start(
                        out=sa_tiles[i][:],
                        in_=sa[d_idx, base : base + P * 2 * w].rearrange(
                            "(p c) -> p c", p=P
                        ),
                    ).then_inc(load_sems[i], 16)
            sync.wait_ge(dve_pipe, 4 * nt * repeat)
            sync.wait_ge(act_pipe, 2 * nt * repeat)
            sync.dma_start(out=out[:], in_=acc[:]).then_inc(st_sem, 16)
            sync.wait_ge(st_sem, 16)

        @block.vector
        def _(vector):
            for kind, r, i in dve_order():
                w = widths[i]
                wt = w_tiles[i]
                if kind == "ttr":
                    vector.wait_ge(load_sems[i], 16 * (r + 1))
                    if r > 0:
                        # w_tiles[i] free once ln(r-1, i) is done
                        vector.wait_ge(act_pipe, ln_seq[r - 1, i])
                    # d = (s * 1) - a, fused per-partition Σd
                    vector.scalar_tensor_tensor(
                        out=wt[:],
                        in0=sa_tiles[i][:, 0:w],
                        scalar=1.0,
                        in1=sa_tiles[i][:, w : 2 * w],
                        op0=ALU.mult,
                        op1=ALU.subtract,
                        accum_out=acc[:, i : i + 1],
                    ).then_inc(dve_pipe, 1)
                elif kind == "ts":
                    # v = 1 + u (u written by ACT exp)
                    vector.wait_ge(act_pipe, exp_seq[r, i])
                    vector.tensor_scalar_add(
                        out=wt[:], in0=wt[:], scalar1=1.0
                    ).then_inc(dve_pipe, 1)
                elif kind == "p":
                    vector.wait_ge(dve_pipe, ts_seq[r, i])
                    vector.tensor_mul(
                        out=wt[:, 0 : w // 2],
                        in0=wt[:, 0 : w // 2],
                        in1=wt[:, w // 2 : w],
                    ).then_inc(dve_pipe, 1)
                else:  # q
                    vector.wait_ge(dve_pipe, p_seq[r, i])
                    vector.tensor_mul(
                        out=wt[:, 0 : w // 4],
                        in0=wt[:, 0 : w // 4],
                        in1=wt[:, w // 4 : w // 2],
                    ).then_inc(dve_pipe, 1)

        @block.scalar
        def _(scalar):
            for kind, r, i in act_order():
                w = widths[i]
                wt = w_tiles[i]
                if kind == "exp":
                    # u = exp(-d); d ready once ttr(r, i) is done
                    scalar.wait_ge(dve_pipe, ttr_seq[r, i])
                    scalar.activation(
                        out=wt[:],
                        in_=wt[:],
                        func=mybir.ActivationFunctionType.Exp,
                        scale=-1.0,
                    ).then_inc(act_pipe, 1)
                else:  # ln: Σln(q) over the quarter-width products
                    scalar.wait_ge(dve_pipe, q_seq[r, i])
                    scalar.activation(
                        out=wt[:, 0 : w // 4],
                        in_=wt[:, 0 : w // 4],
                        func=mybir.ActivationFunctionType.Ln,
                        accum_out=acc[:, nt + i : nt + i + 1],
                    ).then_inc(act_pipe, 1)

    _nc_cache[key] = nc
    return nc


def _pack_sa(synonymy_score, antonymy_score, widths=WIDTHS):
    """Per core: flat [1, P*2*WTOT] bf16 of contiguous per-tile [P, 2w]
    slabs, each row of a slab holding the syn chunk then the ant chunk."""
    syn = np.asarray(synonymy_score, dtype=np.float32).reshape(N_CORES, P, WTOT)
    ant = np.asarray(antonymy_score, dtype=np.float32).reshape(N_CORES, P, WTOT)
    syn = syn.astype(ml_dtypes.bfloat16)
    ant = ant.astype(ml_dtypes.bfloat16)
    sa = np.empty((N_CORES, 1, P * 2 * WTOT), dtype=ml_dtypes.bfloat16)
    off = 0
    for w in widths:
        blk = np.concatenate(
            [syn[:, :, off : off + w], ant[:, :, off : off + w]], axis=2
        )  # [N_CORES, P, 2w]
        base = P * 2 * off
        sa[:, 0, base : base + P * 2 * w] = blk.reshape(N_CORES, -1)
        off += w
    return sa


def _run(synonymy_score, antonymy_score, **spmd_kwargs):
    nc = _build_nc()
    sa = _pack_sa(synonymy_score, antonymy_score)
    in_maps = [{"sa": sa[c]} for c in range(N_CORES)]
    r = run_bass_kernel_spmd(nc, in_maps, list(range(N_CORES)), **spmd_kwargs)
    nt = len(WIDTHS)
    sum_d = np.float64(0.0)
    sum_ln = np.float64(0.0)
    for c in range(N_CORES):
        partials = r.results[c]["out"].astype(np.float64)
        sum_d += partials[:, 0:nt].sum()
        sum_ln += partials[:, nt : 2 * nt].sum()
    value = np.asarray((2.0 * sum_ln + sum_d) / (2.0 * B), dtype=np.float32)
    return value, r


def kernel(S1_out, synonymy_score, antonymy_score):
    return _run(synonymy_score, antonymy_score)[0]
